# revision 28
# baseline (speedup 1.0000x reference)
"""Fused attention-block kernel for Trainium2, 8-core data-parallel over batch.

v9 final (baseline v2 337us -> ~295us traced):
 - Single z12 pass: z1|z2 via one N=512 matmul per (h,dc); biases ride the
   PSUM->SBUF copies (scalar_tensor_tensor in1) so all BN stats are exact
   E[z^2].  Per-token mean sums for layers 1-3 come from a tiny N=4 wsum
   side-matmul whose PSUM tile accumulates across ALL 32 batches (the PE
   does the batch reduction for free) + host bias-mean constants.
 - z3 pass (N=256) overlaps the z12 stats AllReduce.
 - x1/x2 transposes on the PE (dg-scale fused, tsh via K=128 bst matmul);
   relu rides the PSUM->SBUF copy (ACT for x1T, DVE for x2T).
 - x3 = s3*relu(z3b + tsh3/s3): relu on DVE into its own tile (in-place DVE
   ops measured 3-6x slow), s3 folded into the exp bias (ln s3); softmax
   row-sums via a 1/s3-column ones-matmul.
 - z4 mean sums ride l4_tail's stt accum; q4 squares on ACT.
 - AllReduce discipline (each AR has a ~11us floor and they serialize on
   the cc queue): AR input packs run as ACT accum_out ops so they never sit
   behind the busy DVE queue; the z4 AR is split AR4a (batches 0..15, fired
   mid-attention, absorbs cross-core skew) + AR4b (16..31, floor latency),
   summed locally (AllReduce is linear).  A barrier+warmup AR at t=0 eats
   launch skew under the input DMA.  NOTE: splitting AR12 the same way was
   tried and REGRESSED 60us - an extra AR adds a serial link to the cc
   chain; only split when the tail AR would otherwise pay accumulated skew.
 - No gpsimd elementwise (measured ~4us/op), no DMA transposes (corrupt
   batches when two HWDGE queues run them concurrently; 1.2us each when
   serialized on one queue - slower than PE transposes).

Hardcoded: B=256, N=256, D=256, 8 cores -> 32 batches (8192 tokens) per core.
"""
import sys
import types

sys.path.insert(0, "/opt/trn_rl_repo")

import numpy as np
import ml_dtypes
from contextlib import ExitStack

import concourse.bass as bass
import concourse.mybir as mybir
import concourse.tile as tile
from concourse.masks import make_identity

BF16 = mybir.dt.bfloat16
F32 = mybir.dt.float32
NCORES = 8
B_LOC = 32          # batches per core
T = B_LOC * 256     # tokens per core
EPS = 1e-5
AL = mybir.AluOpType
ACT = mybir.ActivationFunctionType
NORM = 1.0 / (NCORES * B_LOC * 256)


def _install_profile_shim():
    """run_bass_kernel_spmd(trace=True) under axon needs antenv.axon_hooks,
    which this image lacks; synthesize it (harmless if tracing unused)."""
    if "antenv.axon_hooks" in sys.modules:
        return
    try:
        import antenv
        mod = types.ModuleType("antenv.axon_hooks")
        mod._hook = None
        mod.set_axon_ntff_profile_hook = lambda h: setattr(mod, "_hook", h)
        mod.get_axon_ntff_profile_hook = lambda: mod._hook
        sys.modules["antenv.axon_hooks"] = mod
        antenv.axon_hooks = mod
        from trn_agent_boot.trn_boot import _ntff_profile_via_ctypes
        hook = _ntff_profile_via_ctypes("/opt/axon/libaxon_pjrt.so")
        if hook is not None:
            mod.set_axon_ntff_profile_hook(hook)
    except Exception:
        pass


def _legalize_waits(nc, max_waits=1):
    """HW instructions carry one sync-wait slot; walrus rejects instructions
    with too many waits.  Hoist extras onto engine-matched NoOps."""
    for f in nc.m.functions:
        for bb in f.blocks:
            insts = bb.instructions
            new_list = []
            for inst in insts:
                si = inst.sync_info
                if si is not None and len(si.on_wait) > max_waits:
                    waits = list(si.on_wait)
                    extra, keep = waits[:-max_waits], waits[-max_waits:]
                    for j, w in enumerate(extra):
                        nop = mybir.InstNoOp(
                            name=f"{inst.name}-waitnop{j}",
                            engine=inst.engine,
                            ins=[], outs=[],
                            sync_info=mybir.SyncInfo(on_wait=[w], on_update=[]),
                        )
                        nc.register_instruction(nop, overwrite=True)
                        new_list.append(nop)
                    inst.sync_info = mybir.SyncInfo(
                        on_wait=keep, on_update=list(si.on_update))
                new_list.append(inst)
            del insts[:]
            for x in new_list:
                insts.append(x)


def build_program():
    nc = bass.Bass("TRN2", target_bir_lowering=False, debug=False,
                   num_devices=NCORES)

    xT_d = nc.dram_tensor("xT", [128, 2, T], BF16, kind="ExternalInput")
    w12_d = nc.dram_tensor("w12", [128, 2, 512], BF16, kind="ExternalInput")
    w12x_d = nc.dram_tensor("w12x", [128, 2, 4], BF16, kind="ExternalInput")
    w3_d = nc.dram_tensor("w3", [128, 2, 257], BF16, kind="ExternalInput")
    w4_d = nc.dram_tensor("w4", [128, 2, 256], BF16, kind="ExternalInput")
    bb_d = nc.dram_tensor("bb", [128, 2, 1024], BF16, kind="ExternalInput")
    gb_d = nc.dram_tensor("gb", [128, 2, 2], F32, kind="ExternalInput")
    hc_d = nc.dram_tensor("hc", [128, 4], F32, kind="ExternalInput")
    out_d = nc.dram_tensor("out", [T, 256], BF16, kind="ExternalOutput")

    groups = [list(range(NCORES))]
    out_r = out_d.ap().rearrange("(b h p) e -> p b h e", b=B_LOC, h=2, p=128)

    with ExitStack() as ctx:
        tc = ctx.enter_context(tile.TileContext(nc))
        big = ctx.enter_context(tc.tile_pool(name="big", bufs=1))
        small = ctx.enter_context(tc.tile_pool(name="small", bufs=1))
        stage = ctx.enter_context(tc.tile_pool(name="stage", bufs=3))
        att = ctx.enter_context(tc.tile_pool(name="att", bufs=8))
        dram = ctx.enter_context(tc.tile_pool(name="dram", bufs=1, space="DRAM"))

        # ---- constants ------------------------------------------------------
        w12 = small.tile([128, 2, 512], BF16, tag="w12")
        w12x = small.tile([128, 2, 4], BF16, tag="w12x")
        w3 = small.tile([128, 2, 257], BF16, tag="w3")
        w4 = small.tile([128, 2, 256], BF16, tag="w4")
        # bb[:, h, :]: 0:512 = (b1|b2), 512:768 = b3, 768:1024 = b4
        bbt = small.tile([128, 2, 1024], BF16, tag="bbt")
        gbt = small.tile([128, 2, 2], F32, tag="gbt")
        hct = small.tile([128, 4], F32, tag="hct")
        idn = small.tile([128, 128], BF16, tag="idn")
        make_identity(nc, idn[:])

        # ---- warmup all-reduce: sync cores while input streams in ----------
        wu = small.tile([128, 1], F32, tag="wu")
        nc.vector.memset(wu[:], 0.0)
        wu_i = dram.tile([128, 1], F32, tag="wu_i")
        wu_o = dram.tile([128, 1], F32, tag="wu_o")
        nc.sync.dma_start(out=wu_i[:], in_=wu[:])
        nc.gpsimd.collective_compute(
            "AllReduce", AL.add, replica_groups=groups,
            ins=[wu_i[:].opt()], outs=[wu_o[:].opt()])

        # ---- xT load: small first chunk, then consts, then the rest --------
        xT = big.tile([128, 2, T], BF16, tag="tpX")
        nc.sync.dma_start(out=xT[:, :, 0:512], in_=xT_d.ap()[:, :, 0:512])
        nc.sync.dma_start(out=w12[:], in_=w12_d.ap())
        nc.sync.dma_start(out=w12x[:], in_=w12x_d.ap())
        nc.sync.dma_start(out=bbt[:], in_=bb_d.ap())
        nc.sync.dma_start(out=w3[:], in_=w3_d.ap())
        nc.sync.dma_start(out=w4[:], in_=w4_d.ap())
        nc.sync.dma_start(out=gbt[:], in_=gb_d.ap())
        nc.sync.dma_start(out=hct[:], in_=hc_d.ap())
        for c in range(8):
            t0, t1 = 512 + c * 1024, min(512 + (c + 1) * 1024, T)
            if t0 < t1:
                nc.sync.dma_start(out=xT[:, :, t0:t1],
                                  in_=xT_d.ap()[:, :, t0:t1])

        # ---- big sbuf tiles -------------------------------------------------
        z1sb = big.tile([128, B_LOC, 2, 256], BF16, tag="tpA")
        z2sb = big.tile([128, B_LOC, 2, 256], BF16, tag="tpC")
        z3sb = big.tile([128, B_LOC, 2, 256], BF16, tag="tpB")
        x2T = big.tile([128, 2, T], BF16, tag="tpE")
        x1T = big.tile([128, 2, T], BF16, tag="tpX")        # aliases xT
        z4sb = big.tile([128, B_LOC, 2, 256], BF16, tag="tpA")  # aliases z1sb
        x3r = big.tile([128, B_LOC, 2, 256], BF16, tag="tpC")   # aliases z2sb
        scrA = big.tile([128, 8, 256], BF16, tag="scrA")

        macc = small.tile([128, 2, B_LOC], F32, tag="macc")      # l4 means
        qacc = small.tile([128, 4, 2, 4], F32, tag="qacc")       # sq sums
        GRP = 8

        def emit_allreduce(lbl, arin, width):
            ar_i = dram.tile([128, width], F32, tag=f"ari{lbl}", name=f"ai{lbl}")
            ar_o = dram.tile([128, width], F32, tag=f"aro{lbl}", name=f"ao{lbl}")
            nc.sync.dma_start(out=ar_i[:], in_=arin[:])
            nc.gpsimd.collective_compute(
                "AllReduce", AL.add, replica_groups=groups,
                ins=[ar_i[:].opt()], outs=[ar_o[:].opt()])
            artot = small.tile([128, width], F32, tag=f"art{lbl}",
                               name=f"at{lbl}")
            nc.sync.dma_start(out=artot[:], in_=ar_o[:])
            return artot

        dmyz = small.tile([128, 32], F32, tag="dmyz")

        # ---- pass A: z1|z2 matmuls (N=512) + wsum side-matmul (N=2) --------
        zpA_cm = tc.tile_pool(name="zpA", bufs=1, space="PSUM")
        zpA = zpA_cm.__enter__()
        pzx = zpA.tile([128, 2, 4], F32, tag="pzx", bufs=1, name="pzx")
        for b in range(B_LOC):
            pz = zpA.tile([128, 2, 512], F32, tag="pz", bufs=2, name=f"pz{b}")
            for h in range(2):
                for dc in range(2):
                    lhs = xT[:, dc, b * 256 + h * 128:b * 256 + (h + 1) * 128]
                    nc.tensor.matmul(out=pz[:, h, :], lhsT=lhs,
                                     rhs=w12[:, dc, :],
                                     start=(dc == 0), stop=(dc == 1))
                    # wsum side-matmul accumulates over ALL batches in PSUM:
                    # pzx[p, h, l] = sum_b sum_e y_l for token (h, p)
                    nc.tensor.matmul(out=pzx[:, h, :], lhsT=lhs,
                                     rhs=w12x[:, dc, :],
                                     start=(b == 0 and dc == 0),
                                     stop=(b == B_LOC - 1 and dc == 1))
            nc.vector.scalar_tensor_tensor(
                out=z1sb[:, b, :, :], in0=pz[:, :, 0:256], scalar=0.0,
                in1=bbt[:, :, 0:256], op0=AL.add, op1=AL.add)
            nc.vector.scalar_tensor_tensor(
                out=z2sb[:, b, :, :], in0=pz[:, :, 256:512], scalar=0.0,
                in1=bbt[:, :, 256:512], op0=AL.add, op1=AL.add)
            if (b + 1) % GRP == 0:
                g = b // GRP
                gs = g * GRP
                for h in range(2):
                    nc.scalar.activation(
                        out=scrA[:], in_=z1sb[:, gs:gs + GRP, h, :],
                        func=ACT.Square, accum_out=qacc[:, 0, h, g:g + 1])
                    nc.scalar.activation(
                        out=scrA[:], in_=z2sb[:, gs:gs + GRP, h, :],
                        func=ACT.Square, accum_out=qacc[:, 1, h, g:g + 1])
        arin12 = small.tile([128, 10], F32, tag="an12")
        nc.scalar.copy(out=arin12[:, 8:10], in_=pzx[:, :, 2])
        for l in range(2):
            for h in range(2):
                nc.scalar.copy(out=arin12[:, 4 * l + 2 * h:4 * l + 2 * h + 1],
                               in_=pzx[:, h, l:l + 1])
                nc.scalar.activation(
                    out=dmyz[:, 0:4], in_=qacc[:, l, h, :],
                    func=ACT.Identity,
                    accum_out=arin12[:, 4 * l + 2 * h + 1:4 * l + 2 * h + 2])
        zpA_cm.__exit__(None, None, None)

        artot12 = emit_allreduce("12", arin12, 10)


        # ---- pass B: z3 (N=257, col 256 = wsum3) ----------------------------
        zpB_cm = tc.tile_pool(name="zpB", bufs=1, space="PSUM")
        zpB = zpB_cm.__enter__()
        for b in range(B_LOC):
            pz3 = zpB.tile([128, 2, 256], F32, tag="pz3", bufs=2, name=f"p3{b}")
            for h in range(2):
                for dc in range(2):
                    nc.tensor.matmul(
                        out=pz3[:, h, :],
                        lhsT=xT[:, dc, b * 256 + h * 128:b * 256 + (h + 1) * 128],
                        rhs=w3[:, dc, 0:256],
                        start=(dc == 0), stop=(dc == 1))
            nc.vector.scalar_tensor_tensor(
                out=z3sb[:, b, :, :], in0=pz3[:, :, 0:256], scalar=0.0,
                in1=bbt[:, :, 512:768], op0=AL.add, op1=AL.add)
            if (b + 1) % GRP == 0:
                g = b // GRP
                gs = g * GRP
                for h in range(2):
                    nc.scalar.activation(
                        out=scrA[:], in_=z3sb[:, gs:gs + GRP, h, :],
                        func=ACT.Square, accum_out=qacc[:, 2, h, g:g + 1])
        zpB_cm.__exit__(None, None, None)

        arin3 = small.tile([128, 2], F32, tag="an3")
        for h in range(2):
            nc.scalar.activation(
                out=dmyz[:, 0:4], in_=qacc[:, 2, h, :], func=ACT.Identity,
                accum_out=arin3[:, h:h + 1])
        artot3 = emit_allreduce("3", arin3, 2)

        # ---- BN finalize ----------------------------------------------------
        def bn_finalize(lbl, artot, off, hc_idx, mean_from_acc=False,
                        q_ap=None):
            mean = small.tile([128, 2], F32, tag=f"mn{lbl}", name=f"mn{lbl}")
            ey2 = small.tile([128, 2], F32, tag=f"ey{lbl}", name=f"ey{lbl}")
            if q_ap is not None:
                # mean cols contiguous at off, q from a separate tensor
                nc.vector.tensor_scalar(mean[:], artot[:, off:off + 2],
                                        NORM, hct[:, hc_idx:hc_idx + 1],
                                        AL.mult, AL.add)
                nc.vector.tensor_scalar_mul(ey2[:], q_ap, NORM)
            elif mean_from_acc:
                nc.vector.tensor_scalar_mul(mean[:], artot[:, off:off + 4:2],
                                            NORM)
                nc.vector.tensor_scalar_mul(ey2[:],
                                            artot[:, off + 1:off + 4:2], NORM)
            else:
                # mean = wsum-sums*NORM + mean(b_l)
                nc.vector.tensor_scalar(mean[:], artot[:, off:off + 4:2],
                                        NORM, hct[:, hc_idx:hc_idx + 1],
                                        AL.mult, AL.add)
                nc.vector.tensor_scalar_mul(ey2[:],
                                            artot[:, off + 1:off + 4:2], NORM)
            var = small.tile([128, 2], F32, tag=f"vr{lbl}", name=f"vr{lbl}")
            nc.vector.tensor_tensor(out=var[:], in0=mean[:], in1=mean[:],
                                    op=AL.mult)
            nc.vector.tensor_tensor(out=var[:], in0=ey2[:], in1=var[:],
                                    op=AL.subtract)
            nc.vector.tensor_scalar_add(var[:], var[:], EPS)
            sd = small.tile([128, 2], F32, tag=f"sd{lbl}", name=f"sd{lbl}")
            nc.scalar.sqrt(out=sd[:], in_=var[:])
            rstd = small.tile([128, 2], F32, tag=f"rs{lbl}", name=f"rs{lbl}")
            nc.vector.reciprocal(out=rstd[:], in_=sd[:])
            s = small.tile([128, 2], F32, tag=f"s{lbl}", name=f"s{lbl}")
            nc.vector.tensor_tensor(out=s[:], in0=rstd[:], in1=gbt[:, :, 0],
                                    op=AL.mult)
            tsh = small.tile([128, 2], F32, tag=f"t{lbl}", name=f"t{lbl}")
            nc.vector.tensor_tensor(out=tsh[:], in0=mean[:], in1=s[:],
                                    op=AL.mult)
            nc.vector.tensor_tensor(out=tsh[:], in0=gbt[:, :, 1], in1=tsh[:],
                                    op=AL.subtract)
            return s, tsh

        s1, tsh1 = bn_finalize("1", artot12, 0, 0)
        s2, tsh2 = bn_finalize("2", artot12, 4, 1)
        s3, tsh3 = bn_finalize("3", artot12, 8, 2, q_ap=artot3[:, 0:2])

        # dg diag(s) for the PE transposes; c1/c2 = tsh/s for the pre-relu
        # (s > 0 since gamma > 0: relu(s*z + tsh) = s * relu(z + tsh/s))
        dg = small.tile([128, 2, 2, 128], BF16, tag="dg")
        c12 = small.tile([128, 2, 2], F32, tag="c12")
        s12i = small.tile([128, 2, 2], F32, tag="s12i")
        for l, s_l, tsh_l in ((0, s1, tsh1), (1, s2, tsh2)):
            for h in range(2):
                nc.vector.tensor_scalar_mul(dg[:, l, h, :], idn[:],
                                            s_l[:, h:h + 1])
            nc.vector.reciprocal(out=s12i[:, l, :], in_=s_l[:])
            nc.vector.tensor_tensor(out=c12[:, l, :], in0=tsh_l[:],
                                    in1=s12i[:, l, :], op=AL.mult)

        # x3 helpers: c3 = tsh3/s3, lns3 = ln(s3), s3i column (bf16)
        s3i = small.tile([128, 2], F32, tag="s3i")
        nc.vector.reciprocal(out=s3i[:], in_=s3[:])
        c3 = small.tile([128, 2], F32, tag="c3")
        nc.vector.tensor_tensor(out=c3[:], in0=tsh3[:], in1=s3i[:], op=AL.mult)
        lns3 = small.tile([128, 2], F32, tag="lns3")
        nc.scalar.activation(out=lns3[:], in_=s3[:], func=ACT.Ln)
        s3ib = small.tile([128, 2], BF16, tag="s3ib")
        nc.vector.tensor_scalar_add(s3ib[:], s3i[:], 0.0)

        # ---- transpose passes (PE): x1T on ACT-relu, x2T on DVE-max --------
        tp_cm = tc.tile_pool(name="tp", bufs=1, space="PSUM")
        tp = tp_cm.__enter__()

        def t_pass(b):
            # pre-relu in token layout (per-partition bias), dg-scale rides
            # the transpose matmul, plain PSUM->SBUF copies after
            u1 = att.tile([128, 2, 256], BF16, tag="u1", bufs=4, name=f"u1_{b}")
            u2 = att.tile([128, 2, 256], BF16, tag="u2", bufs=4, name=f"u2_{b}")
            for h in range(2):
                nc.scalar.activation(
                    out=u1[:, h, :], in_=z1sb[:, b, h, :], func=ACT.Relu,
                    bias=c12[:, 0, h:h + 1])
                nc.vector.tensor_scalar(u2[:, h, :], z2sb[:, b, h, :],
                                        c12[:, 1, h:h + 1], 0.0,
                                        AL.add, AL.max)
            for l, usb, xiT in ((0, u1, x1T), (1, u2, x2T)):
                pst = tp.tile([128, 2, 2, 128], F32, tag="pst", bufs=2,
                              name=f"pt{l}_{b}")
                for h in range(2):
                    for dc in range(2):
                        nc.tensor.matmul(
                            out=pst[:, dc, h, :],
                            lhsT=usb[:, h, dc * 128:(dc + 1) * 128],
                            rhs=dg[:, l, h, :],
                            start=True, stop=True)
                src = pst[:].rearrange("p dc h t -> p dc (h t)")
                if l == 0:
                    nc.scalar.copy(
                        out=xiT[:, :, b * 256:(b + 1) * 256], in_=src)
                else:
                    nc.vector.tensor_scalar_add(
                        xiT[:, :, b * 256:(b + 1) * 256], src, 0.0)

        def x3a_op(b):
            # relu3 = relu(z3b + c3) into x3r (not in place)
            for h in range(2):
                nc.vector.tensor_scalar(x3r[:, b, h, :], z3sb[:, b, h, :],
                                        c3[:, h:h + 1], 0.0, AL.add, AL.max)

        # ---- attention + L4 -------------------------------------------------
        invrc = small.tile([128, B_LOC, 2], F32, tag="invrc")
        b4e = small.tile([128, 2, 256], BF16, tag="b4e")
        nc.vector.tensor_scalar_add(b4e[:], bbt[:, :, 768:1024], 0.0)

        ap_cm = tc.tile_pool(name="ap", bufs=1, space="PSUM")
        ap = ap_cm.__enter__()
        pinv_cm = tc.tile_pool(name="pinvp", bufs=1, space="PSUM")
        pinvp = pinv_cm.__enter__()

        rts = {}
        psys = {}
        pts = {}

        def s_exp(b):
            pss = ap.tile([128, 2, 256], F32, tag="pss", bufs=2, name=f"ps{b}")
            for mc in range(2):
                for ec in range(2):
                    nc.tensor.matmul(
                        out=pss[:, mc, :],
                        lhsT=x2T[:, ec, b * 256 + mc * 128:b * 256 + (mc + 1) * 128],
                        rhs=x1T[:, ec, b * 256:(b + 1) * 256],
                        start=(ec == 0), stop=(ec == 1))
            pt = att.tile([128, 2, 256], BF16, tag="pt", bufs=6, name=f"pt{b}")
            for mc in range(2):
                nc.scalar.activation(out=pt[:, mc, :], in_=pss[:, mc, :],
                                     scale=1.0 / 16.0, bias=lns3[:, mc:mc + 1],
                                     func=ACT.Exp)
            pts[b] = pt

        def av(b):
            pt = pts.pop(b)
            prt = ap.tile([128, 2, 256], F32, tag="pq", bufs=2, name=f"pr{b}")
            for dc in range(2):
                for mc in range(2):
                    nc.tensor.matmul(
                        out=prt[:, dc, :],
                        lhsT=x3r[:, b, mc, dc * 128:(dc + 1) * 128],
                        rhs=pt[:, mc, :],
                        start=(mc == 0), stop=(mc == 1))
            pinv = pinvp.tile([128, 2], F32, tag="pinv", bufs=2, name=f"pi{b}")
            for nc_ in range(2):
                for mc in range(2):
                    nc.tensor.matmul(
                        out=pinv[:, nc_:nc_ + 1],
                        lhsT=pt[:, mc, nc_ * 128:(nc_ + 1) * 128],
                        rhs=s3ib[:, mc:mc + 1],
                        start=(mc == 0), stop=(mc == 1))
            nc.vector.reciprocal(out=invrc[:, b, :], in_=pinv[:])
            rT = att.tile([128, 2, 256], BF16, tag="rT", bufs=6, name=f"rT{b}")
            nc.scalar.copy(out=rT[:, 0, :], in_=prt[:, 0, :])
            nc.vector.tensor_scalar_add(rT[:, 1, :], prt[:, 1, :], 0.0)
            rts[b] = rT

        def l4(b):
            psy = ap.tile([128, 2, 256], F32, tag="pq", bufs=2, name=f"py{b}")
            rt = rts.pop(b)
            for h in range(2):
                for dc in range(2):
                    nc.tensor.matmul(
                        out=psy[:, h, :],
                        lhsT=rt[:, dc, h * 128:(h + 1) * 128],
                        rhs=w4[:, dc, :],
                        start=(dc == 0), stop=(dc == 1))
            psys[b] = psy

        def l4_tail(b):
            psy = psys.pop(b)
            for h in range(2):
                nc.vector.scalar_tensor_tensor(
                    out=z4sb[:, b, h, :], in0=psy[:, h, :],
                    scalar=invrc[:, b, h:h + 1], in1=b4e[:, h, :],
                    op0=AL.mult, op1=AL.add,
                    accum_out=macc[:, h, b:b + 1])

        def z4_stats(g):
            gs = g * GRP
            for h in range(2):
                nc.scalar.activation(
                    out=scrA[:], in_=z4sb[:, gs:gs + GRP, h, :],
                    func=ACT.Square, accum_out=qacc[:, 3, h, g:g + 1])

        # pipeline
        t_pass(0)
        t_pass(1)
        x3a_op(0)
        s_exp(0)
        for b in range(B_LOC):
            av(b)
            if b >= 1:
                l4(b - 1)
            if b >= 2:
                l4_tail(b - 2)
            if b + 1 < B_LOC:
                s_exp(b + 1)
                x3a_op(b + 1)
            if b + 2 < B_LOC:
                t_pass(b + 2)
            if b == 18:
                z4_stats(0)
                z4_stats(1)
                ar4a_in = small.tile([128, 4], F32, tag="an4a")
                dmy = small.tile([128, 16], F32, tag="dmy4a")
                for h in range(2):
                    nc.scalar.activation(
                        out=dmy[:], in_=macc[:, h, 0:16], func=ACT.Identity,
                        accum_out=ar4a_in[:, 2 * h:2 * h + 1])
                    nc.scalar.activation(
                        out=dmy[:, 0:2], in_=qacc[:, 3, h, 0:2],
                        func=ACT.Identity,
                        accum_out=ar4a_in[:, 2 * h + 1:2 * h + 2])
                artot4a = emit_allreduce("4a", ar4a_in, 4)
            if b == 26:
                z4_stats(2)
        l4(B_LOC - 1)
        l4_tail(B_LOC - 2)
        l4_tail(B_LOC - 1)

        # ---- tail stats: batches 16..31 ------------------------------------
        z4_stats(3)
        ar4b_in = small.tile([128, 4], F32, tag="an4b")
        dmyb = small.tile([128, 16], F32, tag="dmy4b")
        for h in range(2):
            nc.scalar.activation(
                out=dmyb[:], in_=macc[:, h, 16:B_LOC], func=ACT.Identity,
                accum_out=ar4b_in[:, 2 * h:2 * h + 1])
            nc.scalar.activation(
                out=dmyb[:, 0:2], in_=qacc[:, 3, h, 2:4], func=ACT.Identity,
                accum_out=ar4b_in[:, 2 * h + 1:2 * h + 2])
        artot4b = emit_allreduce("4b", ar4b_in, 4)
        artot4 = small.tile([128, 4], F32, tag="art4")
        nc.vector.tensor_tensor(out=artot4[:], in0=artot4a[:], in1=artot4b[:],
                                op=AL.add)
        s4, tsh4 = bn_finalize("4", artot4, 0, 3, mean_from_acc=True)
        c4 = small.tile([128, 2], F32, tag="c4")
        s4i = small.tile([128, 2], F32, tag="s4i")
        nc.vector.reciprocal(out=s4i[:], in_=s4[:])
        nc.vector.tensor_tensor(out=c4[:], in0=tsh4[:], in1=s4i[:], op=AL.mult)

        # ---- final BN+relu: ACT h0, DVE h1; DMA out per batch --------------
        for b in range(0, B_LOC, 2):
            orl = stage.tile([128, 2, 2, 256], BF16, tag="orl", name=f"or{b}")
            tmp = stage.tile([128, 2, 256], BF16, tag="tmpf", name=f"tf{b}")
            # h0 of both batches on ACT (fused relu+scale+bias)
            nc.scalar.activation(
                out=orl[:, :, 0, :], in_=z4sb[:, b:b + 2, 0, :], func=ACT.Relu,
                scale=s4[:, 0:1], bias=tsh4[:, 0:1])
            # h1 of both batches on DVE (2 fused tensor_scalar ops)
            nc.vector.tensor_scalar(tmp[:], z4sb[:, b:b + 2, 1, :],
                                    c4[:, 1:2], 0.0, AL.add, AL.max)
            nc.vector.tensor_scalar_mul(orl[:, :, 1, :], tmp[:], s4[:, 1:2])
            nc.sync.dma_start(out=out_r[:, b:b + 2, :, :], in_=orl[:])

        pinv_cm.__exit__(None, None, None)
        ap_cm.__exit__(None, None, None)
        tp_cm.__exit__(None, None, None)

    _legalize_waits(nc)
    return nc


_CACHE = {}


def _prep_core_inputs(inputs):
    bf = ml_dtypes.bfloat16
    W = [inputs["W1"], inputs["W2"], inputs["W3"], inputs["W4"]]
    bs = [inputs["b1"], inputs["b2"], inputs["b3"], inputs["b4"]]
    gamma, beta = inputs["gamma"], inputs["beta"]

    w12 = np.zeros((128, 2, 512), dtype=bf)
    w12x = np.zeros((128, 2, 4), dtype=bf)
    w3 = np.zeros((128, 2, 257), dtype=bf)
    w4 = np.zeros((128, 2, 256), dtype=bf)
    for c in range(2):
        w12[:, c, 0:256] = W[0][:, c * 128:(c + 1) * 128].T.astype(bf)
        w12[:, c, 256:512] = W[1][:, c * 128:(c + 1) * 128].T.astype(bf)
        w3[:, c, 0:256] = W[2][:, c * 128:(c + 1) * 128].T.astype(bf)
        w4[:, c, :] = W[3][:, c * 128:(c + 1) * 128].T.astype(bf)
        for l in range(3):
            ws = W[l].astype(np.float64).sum(axis=0).astype(np.float32)
            w12x[:, c, l] = ws[c * 128:(c + 1) * 128].astype(bf)
    bb = np.zeros((128, 2, 1024), dtype=bf)
    for h in range(2):
        bb[:, h, 0:256] = bs[0][None, :].astype(bf)
        bb[:, h, 256:512] = bs[1][None, :].astype(bf)
        bb[:, h, 512:768] = bs[2][None, :].astype(bf)
        bb[:, h, 768:1024] = bs[3][None, :].astype(bf)
    gb = np.zeros((128, 2, 2), dtype=np.float32)
    for h in range(2):
        gb[:, h, 0] = gamma[h * 128:(h + 1) * 128]
        gb[:, h, 1] = beta[h * 128:(h + 1) * 128]
    hc = np.zeros((128, 4), dtype=np.float32)
    for l in range(4):
        hc[:, l] = bs[l].mean(dtype=np.float64)
    return w12, w12x, w3, w4, bb, gb, hc


def kernel(**inputs):
    _install_profile_shim()
    from concourse.bass_utils import run_bass_kernel_spmd

    if "nc" not in _CACHE:
        _CACHE["nc"] = build_program()
    nc = _CACHE["nc"]

    x = np.asarray(inputs["x"], dtype=np.float32)
    w12, w12x, w3, w4, bb, gb, hc = _prep_core_inputs(
        {k: np.asarray(v) for k, v in inputs.items()})

    bf = ml_dtypes.bfloat16
    in_maps = []
    for i in range(NCORES):
        xs = x[i * B_LOC:(i + 1) * B_LOC].reshape(T, 256)
        xTh = np.ascontiguousarray(
            xs.T.reshape(2, 128, T).transpose(1, 0, 2)).astype(bf)
        in_maps.append({"xT": xTh, "w12": w12, "w12x": w12x, "w3": w3,
                       "w4": w4, "bb": bb, "gb": gb, "hc": hc})

    trace = _CACHE.get("trace", False)
    res = run_bass_kernel_spmd(nc, in_maps, list(range(NCORES)), trace=trace)
    _CACHE["last_result"] = res

    out = np.empty((256, 256, 256), dtype=np.float32)
    for i in range(NCORES):
        out[i * B_LOC:(i + 1) * B_LOC] = np.asarray(
            res.results[i]["out"], dtype=np.float32).reshape(B_LOC, 256, 256)
    return out


# revision 29
# speedup vs baseline: 1.0640x; 1.0640x over previous
"""Fused attention-block kernel for Trainium2, 8-core data-parallel over batch.

v9 final (baseline v2 337us -> ~295us traced):
 - Single z12 pass: z1|z2 via one N=512 matmul per (h,dc); biases ride the
   PSUM->SBUF copies (scalar_tensor_tensor in1) so all BN stats are exact
   E[z^2].  Per-token mean sums for layers 1-3 come from a tiny N=4 wsum
   side-matmul whose PSUM tile accumulates across ALL 32 batches (the PE
   does the batch reduction for free) + host bias-mean constants.
 - z3 pass (N=256) overlaps the z12 stats AllReduce.
 - x1/x2 transposes on the PE (dg-scale fused, tsh via K=128 bst matmul);
   relu rides the PSUM->SBUF copy (ACT for x1T, DVE for x2T).
 - x3 = s3*relu(z3b + tsh3/s3): relu on DVE into its own tile (in-place DVE
   ops measured 3-6x slow), s3 folded into the exp bias (ln s3); softmax
   row-sums via a 1/s3-column ones-matmul.
 - z4 mean sums ride l4_tail's stt accum; q4 squares on ACT.
 - AllReduce discipline (each AR has a ~11us floor and they serialize on
   the cc queue): AR input packs run as ACT accum_out ops so they never sit
   behind the busy DVE queue; the z4 AR is split AR4a (batches 0..15, fired
   mid-attention, absorbs cross-core skew) + AR4b (16..31, floor latency),
   summed locally (AllReduce is linear).  A barrier+warmup AR at t=0 eats
   launch skew under the input DMA.  NOTE: splitting AR12 the same way was
   tried and REGRESSED 60us - an extra AR adds a serial link to the cc
   chain; only split when the tail AR would otherwise pay accumulated skew.
 - No gpsimd elementwise (measured ~4us/op), no DMA transposes (corrupt
   batches when two HWDGE queues run them concurrently; 1.2us each when
   serialized on one queue - slower than PE transposes).

Hardcoded: B=256, N=256, D=256, 8 cores -> 32 batches (8192 tokens) per core.
"""
import sys
import types

sys.path.insert(0, "/opt/trn_rl_repo")

import numpy as np
import ml_dtypes
from contextlib import ExitStack

import concourse.bass as bass
import concourse.mybir as mybir
import concourse.tile as tile
from concourse.masks import make_identity

BF16 = mybir.dt.bfloat16
F32 = mybir.dt.float32
NCORES = 8
B_LOC = 32          # batches per core
T = B_LOC * 256     # tokens per core
EPS = 1e-5
AL = mybir.AluOpType
ACT = mybir.ActivationFunctionType
NORM = 1.0 / (NCORES * B_LOC * 256)


def _install_profile_shim():
    """run_bass_kernel_spmd(trace=True) under axon needs antenv.axon_hooks,
    which this image lacks; synthesize it (harmless if tracing unused)."""
    if "antenv.axon_hooks" in sys.modules:
        return
    try:
        import antenv
        mod = types.ModuleType("antenv.axon_hooks")
        mod._hook = None
        mod.set_axon_ntff_profile_hook = lambda h: setattr(mod, "_hook", h)
        mod.get_axon_ntff_profile_hook = lambda: mod._hook
        sys.modules["antenv.axon_hooks"] = mod
        antenv.axon_hooks = mod
        from trn_agent_boot.trn_boot import _ntff_profile_via_ctypes
        hook = _ntff_profile_via_ctypes("/opt/axon/libaxon_pjrt.so")
        if hook is not None:
            mod.set_axon_ntff_profile_hook(hook)
    except Exception:
        pass


def _legalize_waits(nc, max_waits=1):
    """HW instructions carry one sync-wait slot; walrus rejects instructions
    with too many waits.  Hoist extras onto engine-matched NoOps."""
    for f in nc.m.functions:
        for bb in f.blocks:
            insts = bb.instructions
            new_list = []
            for inst in insts:
                si = inst.sync_info
                if si is not None and len(si.on_wait) > max_waits:
                    waits = list(si.on_wait)
                    extra, keep = waits[:-max_waits], waits[-max_waits:]
                    for j, w in enumerate(extra):
                        nop = mybir.InstNoOp(
                            name=f"{inst.name}-waitnop{j}",
                            engine=inst.engine,
                            ins=[], outs=[],
                            sync_info=mybir.SyncInfo(on_wait=[w], on_update=[]),
                        )
                        nc.register_instruction(nop, overwrite=True)
                        new_list.append(nop)
                    inst.sync_info = mybir.SyncInfo(
                        on_wait=keep, on_update=list(si.on_update))
                new_list.append(inst)
            del insts[:]
            for x in new_list:
                insts.append(x)


def build_program():
    nc = bass.Bass("TRN2", target_bir_lowering=False, debug=False,
                   num_devices=NCORES)

    xT_d = nc.dram_tensor("xT", [128, 2, T], BF16, kind="ExternalInput")
    w12_d = nc.dram_tensor("w12", [128, 2, 512], BF16, kind="ExternalInput")
    w12x_d = nc.dram_tensor("w12x", [128, 2, 4], BF16, kind="ExternalInput")
    w3_d = nc.dram_tensor("w3", [128, 2, 257], BF16, kind="ExternalInput")
    w4_d = nc.dram_tensor("w4", [128, 2, 256], BF16, kind="ExternalInput")
    bb_d = nc.dram_tensor("bb", [128, 2, 1024], BF16, kind="ExternalInput")
    gb_d = nc.dram_tensor("gb", [128, 2, 2], F32, kind="ExternalInput")
    hc_d = nc.dram_tensor("hc", [128, 4], F32, kind="ExternalInput")
    out_d = nc.dram_tensor("out", [T, 256], BF16, kind="ExternalOutput")

    groups = [list(range(NCORES))]
    out_r = out_d.ap().rearrange("(b h p) e -> p b h e", b=B_LOC, h=2, p=128)

    with ExitStack() as ctx:
        tc = ctx.enter_context(tile.TileContext(nc))
        big = ctx.enter_context(tc.tile_pool(name="big", bufs=1))
        small = ctx.enter_context(tc.tile_pool(name="small", bufs=1))
        stage = ctx.enter_context(tc.tile_pool(name="stage", bufs=3))
        att = ctx.enter_context(tc.tile_pool(name="att", bufs=8))
        dram = ctx.enter_context(tc.tile_pool(name="dram", bufs=1, space="DRAM"))

        # ---- constants ------------------------------------------------------
        w12 = small.tile([128, 2, 512], BF16, tag="w12")
        w12x = small.tile([128, 2, 4], BF16, tag="w12x")
        w3 = small.tile([128, 2, 257], BF16, tag="w3")
        w4 = small.tile([128, 2, 256], BF16, tag="w4")
        # bb[:, h, :]: 0:512 = (b1|b2), 512:768 = b3, 768:1024 = b4
        bbt = small.tile([128, 2, 1024], BF16, tag="bbt")
        gbt = small.tile([128, 2, 2], F32, tag="gbt")
        hct = small.tile([128, 4], F32, tag="hct")
        idn = small.tile([128, 128], BF16, tag="idn")
        make_identity(nc, idn[:])

        # ---- warmup all-reduce: sync cores while input streams in ----------
        wu = small.tile([128, 1], F32, tag="wu")
        nc.vector.memset(wu[:], 0.0)
        wu_i = dram.tile([128, 1], F32, tag="wu_i")
        wu_o = dram.tile([128, 1], F32, tag="wu_o")
        nc.sync.dma_start(out=wu_i[:], in_=wu[:])
        nc.gpsimd.collective_compute(
            "AllReduce", AL.add, replica_groups=groups,
            ins=[wu_i[:].opt()], outs=[wu_o[:].opt()])

        # ---- xT load: small first chunk, then consts, then the rest --------
        xT = big.tile([128, 2, T], BF16, tag="tpX")
        nc.sync.dma_start(out=xT[:, :, 0:512], in_=xT_d.ap()[:, :, 0:512])
        nc.sync.dma_start(out=w12[:], in_=w12_d.ap())
        nc.sync.dma_start(out=w12x[:], in_=w12x_d.ap())
        nc.sync.dma_start(out=bbt[:], in_=bb_d.ap())
        nc.sync.dma_start(out=w3[:], in_=w3_d.ap())
        nc.sync.dma_start(out=w4[:], in_=w4_d.ap())
        nc.sync.dma_start(out=gbt[:], in_=gb_d.ap())
        nc.sync.dma_start(out=hct[:], in_=hc_d.ap())
        for c in range(8):
            t0, t1 = 512 + c * 1024, min(512 + (c + 1) * 1024, T)
            if t0 < t1:
                nc.sync.dma_start(out=xT[:, :, t0:t1],
                                  in_=xT_d.ap()[:, :, t0:t1])

        # ---- big sbuf tiles -------------------------------------------------
        z1sb = big.tile([128, B_LOC, 2, 256], BF16, tag="tpA")
        z2sb = big.tile([128, B_LOC, 2, 256], BF16, tag="tpC")
        z3sb = big.tile([128, B_LOC, 2, 256], BF16, tag="tpB")
        x2T = big.tile([128, 2, T], BF16, tag="tpE")
        x1T = big.tile([128, 2, T], BF16, tag="tpX")        # aliases xT
        z4sb = big.tile([128, B_LOC, 2, 256], BF16, tag="tpA")  # aliases z1sb
        x3r = big.tile([128, B_LOC, 2, 256], BF16, tag="tpC")   # aliases z2sb
        scrA = big.tile([128, 8, 256], BF16, tag="scrA")

        macc = small.tile([128, 2, B_LOC], F32, tag="macc")      # l4 means
        qacc = small.tile([128, 4, 2, 4], F32, tag="qacc")       # sq sums
        GRP = 8

        def emit_allreduce(lbl, arin, width):
            ar_i = dram.tile([128, width], F32, tag=f"ari{lbl}", name=f"ai{lbl}")
            ar_o = dram.tile([128, width], F32, tag=f"aro{lbl}", name=f"ao{lbl}")
            nc.sync.dma_start(out=ar_i[:], in_=arin[:])
            nc.gpsimd.collective_compute(
                "AllReduce", AL.add, replica_groups=groups,
                ins=[ar_i[:].opt()], outs=[ar_o[:].opt()])
            artot = small.tile([128, width], F32, tag=f"art{lbl}",
                               name=f"at{lbl}")
            nc.sync.dma_start(out=artot[:], in_=ar_o[:])
            return artot

        dmyz = small.tile([128, 32], F32, tag="dmyz")

        # ---- pass A: z1|z2 matmuls (N=512) + wsum side-matmul (N=2) --------
        zpA_cm = tc.tile_pool(name="zpA", bufs=1, space="PSUM")
        zpA = zpA_cm.__enter__()
        pzx = zpA.tile([128, 2, 4], F32, tag="pzx", bufs=1, name="pzx")
        for b in range(B_LOC):
            pz = zpA.tile([128, 2, 512], F32, tag="pz", bufs=2, name=f"pz{b}")
            for h in range(2):
                for dc in range(2):
                    lhs = xT[:, dc, b * 256 + h * 128:b * 256 + (h + 1) * 128]
                    nc.tensor.matmul(out=pz[:, h, :], lhsT=lhs,
                                     rhs=w12[:, dc, :],
                                     start=(dc == 0), stop=(dc == 1))
                    # wsum side-matmul accumulates over ALL batches in PSUM:
                    # pzx[p, h, l] = sum_b sum_e y_l for token (h, p)
                    nc.tensor.matmul(out=pzx[:, h, :], lhsT=lhs,
                                     rhs=w12x[:, dc, :],
                                     start=(b == 0 and dc == 0),
                                     stop=(b == B_LOC - 1 and dc == 1))
            nc.vector.scalar_tensor_tensor(
                out=z1sb[:, b, :, :], in0=pz[:, :, 0:256], scalar=0.0,
                in1=bbt[:, :, 0:256], op0=AL.add, op1=AL.add)
            nc.vector.scalar_tensor_tensor(
                out=z2sb[:, b, :, :], in0=pz[:, :, 256:512], scalar=0.0,
                in1=bbt[:, :, 256:512], op0=AL.add, op1=AL.add)
            if (b + 1) % GRP == 0:
                g = b // GRP
                gs = g * GRP
                for h in range(2):
                    nc.scalar.activation(
                        out=scrA[:], in_=z1sb[:, gs:gs + GRP, h, :],
                        func=ACT.Square, accum_out=qacc[:, 0, h, g:g + 1])
                    nc.scalar.activation(
                        out=scrA[:], in_=z2sb[:, gs:gs + GRP, h, :],
                        func=ACT.Square, accum_out=qacc[:, 1, h, g:g + 1])
        arin12 = small.tile([128, 10], F32, tag="an12")
        nc.scalar.copy(out=arin12[:, 8:10], in_=pzx[:, :, 2])
        for l in range(2):
            for h in range(2):
                nc.scalar.copy(out=arin12[:, 4 * l + 2 * h:4 * l + 2 * h + 1],
                               in_=pzx[:, h, l:l + 1])
                nc.scalar.activation(
                    out=dmyz[:, 0:4], in_=qacc[:, l, h, :],
                    func=ACT.Identity,
                    accum_out=arin12[:, 4 * l + 2 * h + 1:4 * l + 2 * h + 2])
        zpA_cm.__exit__(None, None, None)

        artot12 = emit_allreduce("12", arin12, 10)


        # ---- pass B: z3 (N=257, col 256 = wsum3) ----------------------------
        zpB_cm = tc.tile_pool(name="zpB", bufs=1, space="PSUM")
        zpB = zpB_cm.__enter__()
        for b in range(B_LOC):
            pz3 = zpB.tile([128, 2, 256], F32, tag="pz3", bufs=2, name=f"p3{b}")
            for h in range(2):
                for dc in range(2):
                    nc.tensor.matmul(
                        out=pz3[:, h, :],
                        lhsT=xT[:, dc, b * 256 + h * 128:b * 256 + (h + 1) * 128],
                        rhs=w3[:, dc, 0:256],
                        start=(dc == 0), stop=(dc == 1))
            nc.vector.scalar_tensor_tensor(
                out=z3sb[:, b, :, :], in0=pz3[:, :, 0:256], scalar=0.0,
                in1=bbt[:, :, 512:768], op0=AL.add, op1=AL.add)
            if (b + 1) % GRP == 0:
                g = b // GRP
                gs = g * GRP
                for h in range(2):
                    nc.scalar.activation(
                        out=scrA[:], in_=z3sb[:, gs:gs + GRP, h, :],
                        func=ACT.Square, accum_out=qacc[:, 2, h, g:g + 1])
        zpB_cm.__exit__(None, None, None)

        arin3 = small.tile([128, 2], F32, tag="an3")
        for h in range(2):
            nc.scalar.activation(
                out=dmyz[:, 0:4], in_=qacc[:, 2, h, :], func=ACT.Identity,
                accum_out=arin3[:, h:h + 1])
        artot3 = emit_allreduce("3", arin3, 2)

        # ---- BN finalize ----------------------------------------------------
        def bn_finalize(lbl, artot, off, hc_idx, mean_from_acc=False,
                        q_ap=None):
            mean = small.tile([128, 2], F32, tag=f"mn{lbl}", name=f"mn{lbl}")
            ey2 = small.tile([128, 2], F32, tag=f"ey{lbl}", name=f"ey{lbl}")
            if q_ap is not None:
                # mean cols contiguous at off, q from a separate tensor
                nc.vector.tensor_scalar(mean[:], artot[:, off:off + 2],
                                        NORM, hct[:, hc_idx:hc_idx + 1],
                                        AL.mult, AL.add)
                nc.vector.tensor_scalar_mul(ey2[:], q_ap, NORM)
            elif mean_from_acc:
                nc.vector.tensor_scalar_mul(mean[:], artot[:, off:off + 4:2],
                                            NORM)
                nc.vector.tensor_scalar_mul(ey2[:],
                                            artot[:, off + 1:off + 4:2], NORM)
            else:
                # mean = wsum-sums*NORM + mean(b_l)
                nc.vector.tensor_scalar(mean[:], artot[:, off:off + 4:2],
                                        NORM, hct[:, hc_idx:hc_idx + 1],
                                        AL.mult, AL.add)
                nc.vector.tensor_scalar_mul(ey2[:],
                                            artot[:, off + 1:off + 4:2], NORM)
            var = small.tile([128, 2], F32, tag=f"vr{lbl}", name=f"vr{lbl}")
            nc.vector.tensor_tensor(out=var[:], in0=mean[:], in1=mean[:],
                                    op=AL.mult)
            nc.vector.tensor_tensor(out=var[:], in0=ey2[:], in1=var[:],
                                    op=AL.subtract)
            nc.vector.tensor_scalar_add(var[:], var[:], EPS)
            sd = small.tile([128, 2], F32, tag=f"sd{lbl}", name=f"sd{lbl}")
            nc.scalar.sqrt(out=sd[:], in_=var[:])
            rstd = small.tile([128, 2], F32, tag=f"rs{lbl}", name=f"rs{lbl}")
            nc.vector.reciprocal(out=rstd[:], in_=sd[:])
            s = small.tile([128, 2], F32, tag=f"s{lbl}", name=f"s{lbl}")
            nc.vector.tensor_tensor(out=s[:], in0=rstd[:], in1=gbt[:, :, 0],
                                    op=AL.mult)
            tsh = small.tile([128, 2], F32, tag=f"t{lbl}", name=f"t{lbl}")
            nc.vector.tensor_tensor(out=tsh[:], in0=mean[:], in1=s[:],
                                    op=AL.mult)
            nc.vector.tensor_tensor(out=tsh[:], in0=gbt[:, :, 1], in1=tsh[:],
                                    op=AL.subtract)
            return s, tsh

        onesb = small.tile([128, 256], BF16, tag="onesb")
        nc.vector.memset(onesb[:], 1.0)

        s1, tsh1 = bn_finalize("1", artot12, 0, 0)
        s2, tsh2 = bn_finalize("2", artot12, 4, 1)
        s3, tsh3 = bn_finalize("3", artot12, 8, 2, q_ap=artot3[:, 0:2])

        # dg diag(s) and bst (tsh broadcast) for the PE transposes
        dg = small.tile([128, 2, 2, 128], BF16, tag="dg")
        bst = small.tile([128, 2, 2, 256], BF16, tag="bst")
        for l, s_l, tsh_l in ((0, s1, tsh1), (1, s2, tsh2)):
            for h in range(2):
                nc.vector.tensor_scalar_mul(dg[:, l, h, :], idn[:],
                                            s_l[:, h:h + 1])
                nc.vector.tensor_scalar_mul(bst[:, l, h, :], onesb[:],
                                            tsh_l[:, h:h + 1])

        # x3 helpers: c3 = tsh3/s3, lns3 = ln(s3), s3i column (bf16)
        s3i = small.tile([128, 2], F32, tag="s3i")
        nc.vector.reciprocal(out=s3i[:], in_=s3[:])
        c3 = small.tile([128, 2], F32, tag="c3")
        nc.vector.tensor_tensor(out=c3[:], in0=tsh3[:], in1=s3i[:], op=AL.mult)
        lns3 = small.tile([128, 2], F32, tag="lns3")
        nc.scalar.activation(out=lns3[:], in_=s3[:], func=ACT.Ln)
        s3ib = small.tile([128, 2], BF16, tag="s3ib")
        nc.vector.tensor_scalar_add(s3ib[:], s3i[:], 0.0)

        # ---- transpose passes (PE): x1T on ACT-relu, x2T on DVE-max --------
        tp_cm = tc.tile_pool(name="tp", bufs=1, space="PSUM")
        tp = tp_cm.__enter__()

        def t_pass(b):
            for l, zsb, xiT in ((0, z1sb, x1T), (1, z2sb, x2T)):
                pst = tp.tile([128, 2, 2, 128], F32, tag="pst", bufs=2,
                              name=f"pt{l}_{b}")
                for h in range(2):
                    for dc in range(2):
                        nc.tensor.matmul(
                            out=pst[:, dc, h, :],
                            lhsT=zsb[:, b, h, dc * 128:(dc + 1) * 128],
                            rhs=dg[:, l, h, :],
                            start=True, stop=False)
                        nc.tensor.matmul(
                            out=pst[:, dc, h, :],
                            lhsT=bst[:, l, h, dc * 128:(dc + 1) * 128],
                            rhs=idn[:],
                            start=False, stop=True)
                src = pst[:].rearrange("p dc h t -> p dc (h t)")
                if l == 0:
                    nc.scalar.activation(
                        out=xiT[:, :, b * 256:(b + 1) * 256], in_=src,
                        func=ACT.Relu)
                else:
                    nc.vector.tensor_scalar_max(
                        xiT[:, :, b * 256:(b + 1) * 256], src, 0.0)

        def x3a_op(b):
            # relu3 = relu(z3b + c3) into x3r (not in place)
            for h in range(2):
                nc.vector.tensor_scalar(x3r[:, b, h, :], z3sb[:, b, h, :],
                                        c3[:, h:h + 1], 0.0, AL.add, AL.max)

        # ---- attention + L4 -------------------------------------------------
        invrc = small.tile([128, B_LOC, 2], F32, tag="invrc")
        b4e = small.tile([128, 2, 256], BF16, tag="b4e")
        nc.vector.tensor_scalar_add(b4e[:], bbt[:, :, 768:1024], 0.0)

        ap_cm = tc.tile_pool(name="ap", bufs=1, space="PSUM")
        ap = ap_cm.__enter__()
        pinv_cm = tc.tile_pool(name="pinvp", bufs=1, space="PSUM")
        pinvp = pinv_cm.__enter__()

        rts = {}
        psys = {}
        pts = {}

        def s_exp(b):
            pss = ap.tile([128, 2, 256], F32, tag="pss", bufs=2, name=f"ps{b}")
            for mc in range(2):
                for ec in range(2):
                    nc.tensor.matmul(
                        out=pss[:, mc, :],
                        lhsT=x2T[:, ec, b * 256 + mc * 128:b * 256 + (mc + 1) * 128],
                        rhs=x1T[:, ec, b * 256:(b + 1) * 256],
                        start=(ec == 0), stop=(ec == 1))
            pt = att.tile([128, 2, 256], BF16, tag="pt", name=f"pt{b}")
            for mc in range(2):
                nc.scalar.activation(out=pt[:, mc, :], in_=pss[:, mc, :],
                                     scale=1.0 / 16.0, bias=lns3[:, mc:mc + 1],
                                     func=ACT.Exp)
            pts[b] = pt

        def av(b):
            pt = pts.pop(b)
            prt = ap.tile([128, 2, 256], F32, tag="pq", bufs=2, name=f"pr{b}")
            for dc in range(2):
                for mc in range(2):
                    nc.tensor.matmul(
                        out=prt[:, dc, :],
                        lhsT=x3r[:, b, mc, dc * 128:(dc + 1) * 128],
                        rhs=pt[:, mc, :],
                        start=(mc == 0), stop=(mc == 1))
            pinv = pinvp.tile([128, 2], F32, tag="pinv", bufs=2, name=f"pi{b}")
            for nc_ in range(2):
                for mc in range(2):
                    nc.tensor.matmul(
                        out=pinv[:, nc_:nc_ + 1],
                        lhsT=pt[:, mc, nc_ * 128:(nc_ + 1) * 128],
                        rhs=s3ib[:, mc:mc + 1],
                        start=(mc == 0), stop=(mc == 1))
            nc.vector.reciprocal(out=invrc[:, b, :], in_=pinv[:])
            rT = att.tile([128, 2, 256], BF16, tag="rT", name=f"rT{b}")
            nc.scalar.copy(out=rT[:, 0, :], in_=prt[:, 0, :])
            nc.vector.tensor_scalar_add(rT[:, 1, :], prt[:, 1, :], 0.0)
            rts[b] = rT

        def l4(b):
            psy = ap.tile([128, 2, 256], F32, tag="pq", bufs=2, name=f"py{b}")
            rt = rts.pop(b)
            for h in range(2):
                for dc in range(2):
                    nc.tensor.matmul(
                        out=psy[:, h, :],
                        lhsT=rt[:, dc, h * 128:(h + 1) * 128],
                        rhs=w4[:, dc, :],
                        start=(dc == 0), stop=(dc == 1))
            psys[b] = psy

        def l4_tail(b):
            psy = psys.pop(b)
            for h in range(2):
                nc.vector.scalar_tensor_tensor(
                    out=z4sb[:, b, h, :], in0=psy[:, h, :],
                    scalar=invrc[:, b, h:h + 1], in1=b4e[:, h, :],
                    op0=AL.mult, op1=AL.add,
                    accum_out=macc[:, h, b:b + 1])

        def z4_stats(g):
            gs = g * GRP
            for h in range(2):
                nc.scalar.activation(
                    out=scrA[:], in_=z4sb[:, gs:gs + GRP, h, :],
                    func=ACT.Square, accum_out=qacc[:, 3, h, g:g + 1])

        # pipeline
        t_pass(0)
        t_pass(1)
        x3a_op(0)
        s_exp(0)
        for b in range(B_LOC):
            av(b)
            if b >= 1:
                l4(b - 1)
            if b >= 2:
                l4_tail(b - 2)
            if b + 1 < B_LOC:
                s_exp(b + 1)
                x3a_op(b + 1)
            if b + 2 < B_LOC:
                t_pass(b + 2)
            if b == 18:
                z4_stats(0)
                z4_stats(1)
                ar4a_in = small.tile([128, 4], F32, tag="an4a")
                dmy = small.tile([128, 16], F32, tag="dmy4a")
                for h in range(2):
                    nc.scalar.activation(
                        out=dmy[:], in_=macc[:, h, 0:16], func=ACT.Identity,
                        accum_out=ar4a_in[:, 2 * h:2 * h + 1])
                    nc.scalar.activation(
                        out=dmy[:, 0:2], in_=qacc[:, 3, h, 0:2],
                        func=ACT.Identity,
                        accum_out=ar4a_in[:, 2 * h + 1:2 * h + 2])
                artot4a = emit_allreduce("4a", ar4a_in, 4)
            if b == 26:
                z4_stats(2)
        l4(B_LOC - 1)
        l4_tail(B_LOC - 2)
        l4_tail(B_LOC - 1)

        # ---- tail stats: batches 16..31 ------------------------------------
        z4_stats(3)
        ar4b_in = small.tile([128, 4], F32, tag="an4b")
        dmyb = small.tile([128, 16], F32, tag="dmy4b")
        for h in range(2):
            nc.scalar.activation(
                out=dmyb[:], in_=macc[:, h, 16:B_LOC], func=ACT.Identity,
                accum_out=ar4b_in[:, 2 * h:2 * h + 1])
            nc.scalar.activation(
                out=dmyb[:, 0:2], in_=qacc[:, 3, h, 2:4], func=ACT.Identity,
                accum_out=ar4b_in[:, 2 * h + 1:2 * h + 2])
        artot4b = emit_allreduce("4b", ar4b_in, 4)
        artot4 = small.tile([128, 4], F32, tag="art4")
        nc.vector.tensor_tensor(out=artot4[:], in0=artot4a[:], in1=artot4b[:],
                                op=AL.add)
        s4, tsh4 = bn_finalize("4", artot4, 0, 3, mean_from_acc=True)
        c4 = small.tile([128, 2], F32, tag="c4")
        s4i = small.tile([128, 2], F32, tag="s4i")
        nc.vector.reciprocal(out=s4i[:], in_=s4[:])
        nc.vector.tensor_tensor(out=c4[:], in0=tsh4[:], in1=s4i[:], op=AL.mult)

        # ---- final BN+relu: ACT h0, DVE h1; DMA out per batch --------------
        for b in range(0, B_LOC, 2):
            orl = stage.tile([128, 2, 2, 256], BF16, tag="orl", name=f"or{b}")
            tmp = stage.tile([128, 2, 256], BF16, tag="tmpf", name=f"tf{b}")
            # h0 of both batches on ACT (fused relu+scale+bias)
            nc.scalar.activation(
                out=orl[:, :, 0, :], in_=z4sb[:, b:b + 2, 0, :], func=ACT.Relu,
                scale=s4[:, 0:1], bias=tsh4[:, 0:1])
            # h1 of both batches on DVE (2 fused tensor_scalar ops)
            nc.vector.tensor_scalar(tmp[:], z4sb[:, b:b + 2, 1, :],
                                    c4[:, 1:2], 0.0, AL.add, AL.max)
            nc.vector.tensor_scalar_mul(orl[:, :, 1, :], tmp[:], s4[:, 1:2])
            nc.sync.dma_start(out=out_r[:, b:b + 2, :, :], in_=orl[:])

        pinv_cm.__exit__(None, None, None)
        ap_cm.__exit__(None, None, None)
        tp_cm.__exit__(None, None, None)

    _legalize_waits(nc)
    return nc


_CACHE = {}


def _prep_core_inputs(inputs):
    bf = ml_dtypes.bfloat16
    W = [inputs["W1"], inputs["W2"], inputs["W3"], inputs["W4"]]
    bs = [inputs["b1"], inputs["b2"], inputs["b3"], inputs["b4"]]
    gamma, beta = inputs["gamma"], inputs["beta"]

    w12 = np.zeros((128, 2, 512), dtype=bf)
    w12x = np.zeros((128, 2, 4), dtype=bf)
    w3 = np.zeros((128, 2, 257), dtype=bf)
    w4 = np.zeros((128, 2, 256), dtype=bf)
    for c in range(2):
        w12[:, c, 0:256] = W[0][:, c * 128:(c + 1) * 128].T.astype(bf)
        w12[:, c, 256:512] = W[1][:, c * 128:(c + 1) * 128].T.astype(bf)
        w3[:, c, 0:256] = W[2][:, c * 128:(c + 1) * 128].T.astype(bf)
        w4[:, c, :] = W[3][:, c * 128:(c + 1) * 128].T.astype(bf)
        for l in range(3):
            ws = W[l].astype(np.float64).sum(axis=0).astype(np.float32)
            w12x[:, c, l] = ws[c * 128:(c + 1) * 128].astype(bf)
    bb = np.zeros((128, 2, 1024), dtype=bf)
    for h in range(2):
        bb[:, h, 0:256] = bs[0][None, :].astype(bf)
        bb[:, h, 256:512] = bs[1][None, :].astype(bf)
        bb[:, h, 512:768] = bs[2][None, :].astype(bf)
        bb[:, h, 768:1024] = bs[3][None, :].astype(bf)
    gb = np.zeros((128, 2, 2), dtype=np.float32)
    for h in range(2):
        gb[:, h, 0] = gamma[h * 128:(h + 1) * 128]
        gb[:, h, 1] = beta[h * 128:(h + 1) * 128]
    hc = np.zeros((128, 4), dtype=np.float32)
    for l in range(4):
        hc[:, l] = bs[l].mean(dtype=np.float64)
    return w12, w12x, w3, w4, bb, gb, hc


def kernel(**inputs):
    _install_profile_shim()
    from concourse.bass_utils import run_bass_kernel_spmd

    if "nc" not in _CACHE:
        _CACHE["nc"] = build_program()
    nc = _CACHE["nc"]

    x = np.asarray(inputs["x"], dtype=np.float32)
    w12, w12x, w3, w4, bb, gb, hc = _prep_core_inputs(
        {k: np.asarray(v) for k, v in inputs.items()})

    bf = ml_dtypes.bfloat16
    in_maps = []
    for i in range(NCORES):
        xs = x[i * B_LOC:(i + 1) * B_LOC].reshape(T, 256)
        xTh = np.ascontiguousarray(
            xs.T.reshape(2, 128, T).transpose(1, 0, 2)).astype(bf)
        in_maps.append({"xT": xTh, "w12": w12, "w12x": w12x, "w3": w3,
                       "w4": w4, "bb": bb, "gb": gb, "hc": hc})

    trace = _CACHE.get("trace", False)
    res = run_bass_kernel_spmd(nc, in_maps, list(range(NCORES)), trace=trace)
    _CACHE["last_result"] = res

    out = np.empty((256, 256, 256), dtype=np.float32)
    for i in range(NCORES):
        out[i * B_LOC:(i + 1) * B_LOC] = np.asarray(
            res.results[i]["out"], dtype=np.float32).reshape(B_LOC, 256, 256)
    return out
